# revision 7
# baseline (speedup 1.0000x reference)
"""AWD-LSTM (3-layer, T=64, B=48, H=1152, V=32000) on 8 TRN2 NeuronCores.

Sharding: tensor-parallel over the 4H gate dim for the recurrence (each core
owns 144 h-dims = 128-aligned chunk r plus a 16-row piece of chunk 8, so the
gathered transposed hidden state lands on 128-partition tiles with no
repacking), vocab-parallel (4000/core) for the decoder.  One ncfw AllGather
of the three layers' transposed h-slices per wavefront super-step; the
decoder is interleaved into the AllGather latency gaps.  LSTM weights are
SBUF-resident in bf16; cell state and gate math in fp32.
"""

import os

import ml_dtypes
import numpy as np

import concourse.bass as bass
import concourse.mybir as mybir
import concourse.tile as tile
from concourse.bass_utils import run_bass_kernel_spmd
from concourse.vector_clock import ScopedClock

V, E, H, NL = 32000, 400, 1152, 3
T, B = 64, 48
NC = 8
SL = H // NC            # 144 h-dims owned per core
GS = 4 * SL             # 576 gate dims per core
EP = 512                # padded embedding width (bias lane at col 400)
VS = V // NC            # 4000 vocab rows per core
NSS = T + NL - 1        # 66 wavefront super-steps
NKC = H // 128          # 9 K-chunks
DEC_NB, DEC_MB = 8, 6   # decoder: 8 vocab chunks of 500; m-tiles in blocks of 6

F32, BF16, I32 = mybir.dt.float32, mybir.dt.bfloat16, mybir.dt.int32
AF = mybir.ActivationFunctionType


class PatchedTC(tile.TileContext):
    """This walrus build folds at most one sync wait per instruction; split
    the tile-exit drain's waits onto one SP nop each."""

    def _drain_and_barrier(self, tick_clock, wait_clock):
        gc = tick_clock.global_clock
        for i in range(len(gc)):
            if gc[i] > 0:
                nop = self.nc.sync.nop(nofuse=True, hint=f"drain_split_{i}")
                pc = ScopedClock()
                pc.require_at_least(None, i, gc[i])
                wait_clock.add_sem_waits(nop.ins, pc)
        self.nc.sync.drain()
        self.nc.all_engine_barrier()
        popped = self.nc._tile_sem_poison_stack.pop()
        assert popped is self._sem_poison
        self.nc.clear_and_free_semaphores(list(self.sems.allocated().values()))
        self.nc.all_engine_barrier()


def split_excess_waits(nc, max_waits=1):
    """Hoist sync waits beyond max_waits onto preceding same-engine nops."""
    for f in nc.m.functions:
        for bb in f.blocks:
            new_insts = []
            for inst in bb.instructions:
                si = getattr(inst, "sync_info", None)
                if si is not None and si.on_wait and len(si.on_wait) > max_waits:
                    extra = si.on_wait[max_waits:]
                    si.on_wait = si.on_wait[:max_waits]
                    for j in range(0, len(extra), max_waits):
                        new_insts.append(mybir.InstNoOp(
                            name=f"{inst.name}_ws{j}", engine=inst.engine,
                            ins=[], outs=[],
                            sync_info=mybir.SyncInfo(
                                on_wait=extra[j:j + max_waits], on_update=[]),
                        ))
                new_insts.append(inst)
            bb.instructions[:] = new_insts


def _own_dims(r):
    return list(range(128 * r, 128 * r + 128)) + \
        list(range(1024 + 16 * r, 1024 + 16 * r + 16))


def _gate_rows(r):
    d = _own_dims(r)
    return [g * H + i for g in range(4) for i in d]


def build(nc):
    di = {}  # dram tensors (inputs)
    def inp(name, shape, dt):
        di[name] = nc.dram_tensor(name, shape, dt, kind="ExternalInput")
        return di[name]

    x_idx = inp("x_idx", [T * B], I32)
    embw = inp("embw", [V, E], F32)
    wi0 = inp("wi0", [EP, GS], BF16)                  # padded, bias at row 400
    wrec = inp("wrec", [5, H, GS], BF16)              # Wh0,Wi1,Wh1,Wi2,Wh2 (transposed slices)
    bias12 = inp("bias12", [2, B, GS], F32)           # replicated bias tiles l1,l2
    wdec = inp("wdec", [H, VS], BF16)
    bdec = inp("bdec", [128, VS], F32)
    ident = inp("ident", [B, B], F32)
    h0s = inp("h0s", [NL, 128 + 16, B], BF16)         # own transposed h0 slices (main+tail rows)
    c0loc = inp("c0loc", [NL, B, SL], F32)
    h0full = inp("h0full", [NL, H, B], BF16)          # full transposed h0

    dec_out = nc.dram_tensor("dec_out", [T * B, VS], F32, kind="ExternalOutput")
    h_out = nc.dram_tensor("h_out", [NL, B, SL], F32, kind="ExternalOutput")
    c_out = nc.dram_tensor("c_out", [NL, B, SL], F32, kind="ExternalOutput")

    with PatchedTC(nc) as tc:
        with tc.tile_pool(name="wpool", bufs=1) as wp, \
             tc.tile_pool(name="spool", bufs=2) as sp, \
             tc.tile_pool(name="work", bufs=2) as wk, \
             tc.tile_pool(name="psum", bufs=2, space="PSUM") as pp, \
             tc.tile_pool(name="psumt", bufs=2, space="PSUM") as ppt, \
             tc.tile_pool(name="psumd", bufs=1, space="PSUM") as ppd, \
             tc.tile_pool(name="dram", bufs=2, space="DRAM") as dram:

            # ---- persistent SBUF ----
            wi0_sb = wp.tile([128, 4, GS], BF16)
            nc.sync.dma_start(wi0_sb[:], wi0[:].rearrange("(c p) n -> p c n", p=128))
            wrec_sb = wp.tile([128, 5, NKC, GS], BF16)
            nc.sync.dma_start(
                wrec_sb[:], wrec[:].rearrange("w (c p) n -> p w c n", p=128))
            b12_sb = wp.tile([B, 2, GS], F32)
            nc.sync.dma_start(b12_sb[:], bias12[:].rearrange("w b n -> b w n"))
            id_sb = wp.tile([B, B], F32)
            nc.sync.dma_start(id_sb[:], ident[:])
            hist = wp.tile([128, NKC, T * B], BF16)   # gathered h2^T history

            # initial state
            c_st = [sp.tile([B, SL], F32, tag=f"c{l}", name=f"cst{l}") for l in range(NL)]
            for l in range(NL):
                nc.sync.dma_start(c_st[l][:], c0loc[l])
            stage = sp.tile([128, NL, B], BF16)       # AG staging: main rows
            stage_t = sp.tile([16, NL, B], BF16)      # AG staging: 16-row tail
            nc.sync.dma_start(stage[:], h0s[:, 0:128, :].rearrange("l p b -> p l b"))
            nc.sync.dma_start(stage_t[:], h0s[:, 128:144, :].rearrange("l p b -> p l b"))
            hT = [sp.tile([128, 8, B], BF16, tag=f"hT{l}", name=f"hTi{l}") for l in range(NL)]
            c8T = sp.tile([128, NL, B], BF16, tag="c8T")
            for l in range(NL):
                nc.sync.dma_start(
                    hT[l][:], h0full[l, 0:1024, :].rearrange("(c p) b -> p c b", p=128))
            nc.sync.dma_start(c8T[:], h0full[:, 1024:1152, :].rearrange("l p b -> p l b"))

            # ---- phase 1: embedding gather + transpose + P0 ----
            idx_sb = sp.tile([128, 24], I32)
            nc.sync.dma_start(idx_sb[:], x_idx[:].rearrange("(o p) -> p o", p=128))
            embd = dram.tile([T * B, EP], BF16)
            with tc.tile_pool(name="embp", bufs=2) as ep:
                for j in range(24):
                    emb_f = ep.tile([128, EP], F32, tag="embf")
                    nc.gpsimd.memset(emb_f[:, E:EP], 0.0)
                    nc.gpsimd.indirect_dma_start(
                        out=emb_f[:, 0:E], out_offset=None, in_=embw[:],
                        in_offset=bass.IndirectOffsetOnAxis(ap=idx_sb[:, j:j + 1], axis=0))
                    nc.gpsimd.memset(emb_f[:, E:E + 1], 1.0)
                    emb_b = ep.tile([128, EP], BF16, tag="embb")
                    nc.vector.tensor_copy(emb_b[:], emb_f[:])
                    nc.sync.dma_start(
                        embd[:].rearrange("(o p) e -> p o e", p=128)[:, j, :], emb_b[:])
            p0d = dram.tile([T * B, GS], F32)
            with tc.tile_pool(name="embT", bufs=1) as et:
                embT = et.tile([128, 4, T * B], BF16)
                for c in range(4):
                    nc.sync.dma_start_transpose(
                        embT[:, c, :], embd[:, c * 128:(c + 1) * 128])
                for j in range(32):  # 96-token M-tiles
                    psa = pp.tile([96, 512], F32, tag="pA", name="p0a")
                    psb = pp.tile([96, 64], F32, tag="pB", name="p0b")
                    for c in range(4):
                        lhs = embT[:, c, j * 96:(j + 1) * 96]
                        nc.tensor.matmul(psa[:], lhs, wi0_sb[:, c, 0:512],
                                         start=(c == 0), stop=(c == 3))
                        nc.tensor.matmul(psb[:], lhs, wi0_sb[:, c, 512:GS],
                                         start=(c == 0), stop=(c == 3))
                    p0sb = wk.tile([96, GS], F32, tag="p0sb")
                    nc.vector.tensor_copy(p0sb[:, 0:512], psa[:])
                    nc.vector.tensor_copy(p0sb[:, 512:GS], psb[:])
                    nc.sync.dma_start(p0d[j * 96:(j + 1) * 96, :], p0sb[:])

            # ---- decoder pair queue ----
            dec_pairs = []
            for mb in range(4):
                for n in range(DEC_NB):
                    for m in range(mb * DEC_MB, (mb + 1) * DEC_MB):
                        dec_pairs.append((m, n))
            dec_i = 0
            dec_done = [False] * len(dec_pairs)
            wdec_sb = {"n": -1, "tile": None}

            def emit_dec_pair(m, n):
                if wdec_sb["n"] != n or wdec_sb["tile"] is None:
                    wt = wk.tile([128, NKC, 500], BF16, tag="wdec", name="wt")
                    nc.sync.dma_start(
                        wt[:], wdec[:, n * 500:(n + 1) * 500]
                        .rearrange("(c p) v -> p c v", p=128))
                    bt = wk.tile([128, 500], F32, tag="bdect", name="bt")
                    nc.sync.dma_start(bt[:], bdec[:, n * 500:(n + 1) * 500])
                    wdec_sb["n"], wdec_sb["tile"] = n, (wt, bt)
                wt, bt = wdec_sb["tile"]
                psd = ppd.tile([128, 500], F32, tag="pdec")
                for c in range(NKC):
                    nc.tensor.matmul(psd[:], hist[:, c, m * 128:(m + 1) * 128],
                                     wt[:, c, :], start=(c == 0), stop=(c == NKC - 1))
                osb = wk.tile([128, 500], F32, tag="odec")
                nc.vector.tensor_tensor(
                    osb[:], psd[:], bt[:], mybir.AluOpType.add)
                nc.gpsimd.dma_start(
                    dec_out[m * 128:(m + 1) * 128, n * 500:(n + 1) * 500], osb[:])

            # ---- phase 2: wavefront ----
            for s in range(NSS):
                # decoder fill (history complete through t = s-3)
                budget = 4
                scan = dec_i
                while scan < len(dec_pairs) and budget > 0:
                    m, n = dec_pairs[scan]
                    if dec_done[scan] or (m * 128 + 127) // B > s - 3:
                        scan += 1
                        continue
                    emit_dec_pair(m, n)
                    dec_done[scan] = True
                    budget -= 1
                    scan += 1
                while dec_i < len(dec_pairs) and dec_done[dec_i]:
                    dec_i += 1

                for l in range(NL):
                    t = s - l
                    if not (0 <= t < T):
                        continue
                    psa = pp.tile([B, 512], F32, tag="pA")
                    psb = pp.tile([B, 64], F32, tag="pB")
                    if l == 0:
                        srcs = [(0, hT[0], c8T, 0)]
                    else:
                        srcs = [(2 * l - 1, hT[l - 1], c8T, l - 1),
                                (2 * l, hT[l], c8T, l)]
                    for wslot, ht_t, c8_t, c8l in srcs:
                        for c in range(NKC):
                            lhs = ht_t[:, c, :] if c < 8 else c8_t[:, c8l, :]
                            rhs = wrec_sb[:, wslot, c, :]
                            st = (wslot == srcs[0][0]) and (c == 0)
                            last = (wslot == srcs[-1][0]) and (c == NKC - 1)
                            nc.tensor.matmul(
                                psa[:], lhs, rhs[:, 0:512], start=st, stop=last)
                            nc.tensor.matmul(
                                psb[:], lhs, rhs[:, 512:GS], start=st, stop=last)
                    # p_sb = psum + (P0[t] | bias)
                    p_sb = wk.tile([B, GS], F32, tag="psb")
                    if l == 0:
                        p0t = wk.tile([B, GS], F32, tag="p0t")
                        nc.gpsimd.dma_start(p0t[:], p0d[t * B:(t + 1) * B, :])
                        add_src = p0t[:]
                    else:
                        add_src = b12_sb[:, l - 1, :]
                    nc.vector.tensor_tensor(p_sb[:, 0:512], psa[:],
                                            add_src[:, 0:512], mybir.AluOpType.add)
                    nc.vector.tensor_tensor(p_sb[:, 512:GS], psb[:],
                                            add_src[:, 512:GS], mybir.AluOpType.add)
                    # gates: [i | f | o | g] each SL wide
                    ifo = wk.tile([B, 3 * SL], F32, tag="ifo")
                    g_t = wk.tile([B, SL], F32, tag="gg")
                    nc.scalar.activation(ifo[:], p_sb[:, 0:3 * SL], AF.Sigmoid)
                    nc.scalar.activation(g_t[:], p_sb[:, 3 * SL:GS], AF.Tanh)
                    c_new = sp.tile([B, SL], F32, tag=f"c{l}", name=f"cnew{l}")
                    t1 = wk.tile([B, SL], F32, tag="t1")
                    nc.vector.tensor_tensor(t1[:], ifo[:, SL:2 * SL], c_st[l][:],
                                            mybir.AluOpType.mult)
                    t2 = wk.tile([B, SL], F32, tag="t2")
                    nc.vector.tensor_tensor(t2[:], ifo[:, 0:SL], g_t[:],
                                            mybir.AluOpType.mult)
                    nc.vector.tensor_tensor(c_new[:], t1[:], t2[:],
                                            mybir.AluOpType.add)
                    c_st[l] = c_new
                    tc_t = wk.tile([B, SL], F32, tag="tcx")
                    nc.scalar.activation(tc_t[:], c_new[:], AF.Tanh)
                    h_new = wk.tile([B, SL], F32, tag="hn")
                    nc.vector.tensor_tensor(h_new[:], ifo[:, 2 * SL:3 * SL], tc_t[:],
                                            mybir.AluOpType.mult)
                    if t == T - 1:
                        nc.sync.dma_start(h_out[l], h_new[:])
                        nc.sync.dma_start(c_out[l], c_new[:])
                    # transpose own slice -> staging
                    ptr = ppt.tile([128, 2, B], F32, tag="ptr")
                    nc.tensor.transpose(ptr[:, 0, :], h_new[:, 0:128], id_sb[:])
                    nc.tensor.transpose(ptr[0:16, 1, :], h_new[:, 128:SL], id_sb[:])
                    nc.vector.tensor_copy(stage[:, l, :], ptr[:, 0, :])
                    nc.vector.tensor_copy(stage_t[:, l, :], ptr[0:16, 1, :])

                # ---- exchange ----
                agin = dram.tile([NL * SL, B], BF16, tag="agin")
                nc.sync.dma_start(
                    agin[:].rearrange("(l p) b -> l p b", p=SL)[:, 0:128, :]
                    .rearrange("l p b -> p l b"), stage[:])
                nc.sync.dma_start(
                    agin[:].rearrange("(l p) b -> l p b", p=SL)[:, 128:SL, :]
                    .rearrange("l p b -> p l b"), stage_t[:])
                agout = dram.tile([NC * NL * SL, B], BF16, tag="agout")
                nc.gpsimd.collective_compute(
                    "AllGather", mybir.AluOpType.bypass,
                    replica_groups=[list(range(NC))],
                    ins=[agin.opt()], outs=[agout.opt()])
                agv = agout[:].rearrange("(r l p) b -> r l p b", l=NL, p=SL)
                hT = [sp.tile([128, 8, B], BF16, tag=f"hT{l}", name=f"hTn{l}") for l in range(NL)]
                for l in range(NL):
                    nc.sync.dma_start(
                        hT[l][:], agv[:, l, 0:128, :].rearrange("r p b -> p r b"))
                c8s = wk.tile([16, NC, NL, B], BF16, tag="c8s")
                nc.scalar.dma_start(c8s[:], agv[:, :, 128:SL, :].rearrange("r l p b -> p r l b"))
                c8T = sp.tile([128, NL, B], BF16, tag="c8T")
                for r in range(NC):
                    nc.scalar.dma_start(c8T[16 * r:16 * (r + 1), :, :], c8s[:, r, :, :])
                # h2 history append (h2(t2) for t2 = s-2)
                t2 = s - 2
                if 0 <= t2 < T:
                    nc.scalar.dma_start(
                        hist[:, 0:8, t2 * B:(t2 + 1) * B],
                        agv[:, 2, 0:128, :].rearrange("r p b -> p r b"))
                    nc.scalar.dma_start(hist[:, 8, t2 * B:(t2 + 1) * B], c8T[:, 2, :])

            # ---- decoder tail ----
            for i2 in range(len(dec_pairs)):
                if not dec_done[i2]:
                    emit_dec_pair(*dec_pairs[i2])
                    dec_done[i2] = True

    split_excess_waits(nc)
    return di


def _prep_inputs(r, x, h0, c0, emb_W, W_i0, b_i0, W_h0, b_h0,
                 W_i1, b_i1, W_h1, b_h1, W_i2, b_i2, W_h2, b_h2, W_dec, b_dec):
    bf = ml_dtypes.bfloat16
    gr = _gate_rows(r)
    d = _own_dims(r)

    wi0 = np.zeros((EP, GS), np.float32)
    wi0[0:E, :] = W_i0[gr, :].T
    wi0[E, :] = b_i0[gr] + b_h0[gr]

    wrec = np.stack([W_h0[gr, :].T, W_i1[gr, :].T, W_h1[gr, :].T,
                     W_i2[gr, :].T, W_h2[gr, :].T])  # [5, H, GS]

    b1 = (b_i1[gr] + b_h1[gr]).astype(np.float32)
    b2 = (b_i2[gr] + b_h2[gr]).astype(np.float32)
    bias12 = np.stack([np.tile(b1, (B, 1)), np.tile(b2, (B, 1))])

    vs = slice(r * VS, (r + 1) * VS)
    wdec = W_dec[vs, :].T.astype(bf)
    bdec = np.tile(b_dec[vs].astype(np.float32), (128, 1))

    h0T = np.transpose(h0, (0, 2, 1))  # [NL, H, B]
    h0s = h0T[:, d, :].astype(bf)      # [NL, 144, B]
    return {
        "x_idx": x.reshape(-1).astype(np.int32),
        "embw": emb_W.astype(np.float32),
        "wi0": wi0.astype(bf),
        "wrec": wrec.astype(bf),
        "bias12": bias12.astype(np.float32),
        "wdec": wdec,
        "bdec": bdec.astype(np.float32),
        "ident": np.eye(B, dtype=np.float32),
        "h0s": h0s,
        "c0loc": c0[:, :, d].astype(np.float32),
        "h0full": h0T.astype(bf),
    }


def _enable_axon_profiling():
    """Register the NTFF hook (this image's antenv lacks axon_hooks) and stub
    the artifact upload so trace=True works locally."""
    import sys
    import types

    import concourse.bass_utils as bu
    try:
        from trn_agent_boot.trn_boot import _ntff_profile_via_ctypes
    except ImportError:
        return
    hook = _ntff_profile_via_ctypes("/opt/axon/libaxon_pjrt.so")
    mod = types.ModuleType("antenv.axon_hooks")
    mod.get_axon_ntff_profile_hook = lambda: hook
    mod.set_axon_ntff_profile_hook = lambda h: None
    sys.modules["antenv.axon_hooks"] = mod
    bu.upload_artifacts = lambda tmpdir: "file://" + str(tmpdir)


_CACHE = {}


def kernel(**inputs):
    inputs = {k: np.asarray(v) for k, v in inputs.items()}
    x = inputs["x"].astype(np.int32)
    h0, c0 = inputs["h0"], inputs["c0"]

    if "nc" not in _CACHE:
        nc = bass.Bass("TRN2", target_bir_lowering=False, debug=False,
                       num_devices=NC)
        build(nc)
        _CACHE["nc"] = nc
    nc = _CACHE["nc"]

    in_maps = [_prep_inputs(r, x=x, **{k: v for k, v in inputs.items()
                                       if k not in ("x", "h0", "c0")},
                            h0=h0, c0=c0) for r in range(NC)]

    trace = bool(int(os.environ.get("BASS_LSTM_TRACE", "0")))
    if trace:
        _enable_axon_profiling()
    res = run_bass_kernel_spmd(nc, in_maps, core_ids=list(range(NC)),
                               trace=trace)
    if trace and res.exec_time_ns is not None:
        print(f"HW exec time: {res.exec_time_ns} ns")
        _CACHE["exec_time_ns"] = res.exec_time_ns

    decoded = np.empty((T * B, V), np.float32)
    h_fin = np.empty((NL, B, H), np.float32)
    c_fin = np.empty((NL, B, H), np.float32)
    for r in range(NC):
        out = res.results[r]
        decoded[:, r * VS:(r + 1) * VS] = out["dec_out"]
        d = _own_dims(r)
        h_fin[:, :, d] = out["h_out"]
        c_fin[:, :, d] = out["c_out"]
    return decoded.reshape(T, B, V), (h_fin, c_fin)


# revision 8
# speedup vs baseline: 1.0075x; 1.0075x over previous
"""AWD-LSTM (3-layer, T=64, B=48, H=1152, V=32000) on 8 TRN2 NeuronCores.

Sharding: tensor-parallel over the 4H gate dim for the recurrence (each core
owns 144 h-dims = 128-aligned chunk r plus a 16-row piece of chunk 8, so the
gathered transposed hidden state lands on 128-partition tiles with no
repacking), vocab-parallel (4000/core) for the decoder.  One ncfw AllGather
of the three layers' transposed h-slices per wavefront super-step; the
decoder is interleaved into the AllGather latency gaps.  LSTM weights are
SBUF-resident in bf16; cell state and gate math in fp32.
"""

import os

import ml_dtypes
import numpy as np

import concourse.bass as bass
import concourse.mybir as mybir
import concourse.tile as tile
from concourse.bass_utils import run_bass_kernel_spmd
from concourse.vector_clock import ScopedClock

V, E, H, NL = 32000, 400, 1152, 3
T, B = 64, 48
NC = 8
SL = H // NC            # 144 h-dims owned per core
GS = 4 * SL             # 576 gate dims per core
EP = 512                # padded embedding width (bias lane at col 400)
VS = V // NC            # 4000 vocab rows per core
NSS = T + NL - 1        # 66 wavefront super-steps
NKC = H // 128          # 9 K-chunks
DEC_NB, DEC_MB = 8, 6   # decoder: 8 vocab chunks of 500; m-tiles in blocks of 6

F32, BF16, I32 = mybir.dt.float32, mybir.dt.bfloat16, mybir.dt.int32
AF = mybir.ActivationFunctionType


class PatchedTC(tile.TileContext):
    """This walrus build folds at most one sync wait per instruction; split
    the tile-exit drain's waits onto one SP nop each."""

    def _drain_and_barrier(self, tick_clock, wait_clock):
        gc = tick_clock.global_clock
        for i in range(len(gc)):
            if gc[i] > 0:
                nop = self.nc.sync.nop(nofuse=True, hint=f"drain_split_{i}")
                pc = ScopedClock()
                pc.require_at_least(None, i, gc[i])
                wait_clock.add_sem_waits(nop.ins, pc)
        self.nc.sync.drain()
        self.nc.all_engine_barrier()
        popped = self.nc._tile_sem_poison_stack.pop()
        assert popped is self._sem_poison
        self.nc.clear_and_free_semaphores(list(self.sems.allocated().values()))
        self.nc.all_engine_barrier()


def split_excess_waits(nc, max_waits=1):
    """Hoist sync waits beyond max_waits onto preceding same-engine nops."""
    for f in nc.m.functions:
        for bb in f.blocks:
            new_insts = []
            for inst in bb.instructions:
                si = getattr(inst, "sync_info", None)
                if si is not None and si.on_wait and len(si.on_wait) > max_waits:
                    extra = si.on_wait[max_waits:]
                    si.on_wait = si.on_wait[:max_waits]
                    for j in range(0, len(extra), max_waits):
                        new_insts.append(mybir.InstNoOp(
                            name=f"{inst.name}_ws{j}", engine=inst.engine,
                            ins=[], outs=[],
                            sync_info=mybir.SyncInfo(
                                on_wait=extra[j:j + max_waits], on_update=[]),
                        ))
                new_insts.append(inst)
            bb.instructions[:] = new_insts


def _own_dims(r):
    return list(range(128 * r, 128 * r + 128)) + \
        list(range(1024 + 16 * r, 1024 + 16 * r + 16))


def _gate_rows(r):
    d = _own_dims(r)
    return [g * H + i for g in range(4) for i in d]


def build(nc):
    di = {}  # dram tensors (inputs)
    def inp(name, shape, dt):
        di[name] = nc.dram_tensor(name, shape, dt, kind="ExternalInput")
        return di[name]

    x_idx = inp("x_idx", [T * B], I32)
    embw = inp("embw", [V, E], F32)
    wi0 = inp("wi0", [EP, GS], BF16)                  # padded, bias at row 400
    wrec = inp("wrec", [5, H, GS], BF16)              # Wh0,Wi1,Wh1,Wi2,Wh2 (transposed slices)
    bias12 = inp("bias12", [2, B, GS], F32)           # replicated bias tiles l1,l2
    wdec = inp("wdec", [H, VS], BF16)
    bdec = inp("bdec", [128, VS], F32)
    ident = inp("ident", [B, B], F32)
    h0s = inp("h0s", [NL, 128 + 16, B], BF16)         # own transposed h0 slices (main+tail rows)
    c0loc = inp("c0loc", [NL, B, SL], F32)
    h0full = inp("h0full", [NL, H, B], BF16)          # full transposed h0

    dec_out = nc.dram_tensor("dec_out", [T * B, VS], F32, kind="ExternalOutput")
    h_out = nc.dram_tensor("h_out", [NL, B, SL], F32, kind="ExternalOutput")
    c_out = nc.dram_tensor("c_out", [NL, B, SL], F32, kind="ExternalOutput")

    with PatchedTC(nc) as tc:
        with tc.tile_pool(name="wpool", bufs=1) as wp, \
             tc.tile_pool(name="spool", bufs=2) as sp, \
             tc.tile_pool(name="work", bufs=2) as wk, \
             tc.tile_pool(name="psum", bufs=2, space="PSUM") as pp, \
             tc.tile_pool(name="psumt", bufs=2, space="PSUM") as ppt, \
             tc.tile_pool(name="psumd", bufs=1, space="PSUM") as ppd, \
             tc.tile_pool(name="dram", bufs=2, space="DRAM") as dram:

            # ---- persistent SBUF ----
            wi0_sb = wp.tile([128, 4, GS], BF16)
            nc.sync.dma_start(wi0_sb[:], wi0[:].rearrange("(c p) n -> p c n", p=128))
            wrec_sb = wp.tile([128, 5, NKC, GS], BF16)
            nc.sync.dma_start(
                wrec_sb[:], wrec[:].rearrange("w (c p) n -> p w c n", p=128))
            b12_sb = wp.tile([B, 2, GS], F32)
            nc.sync.dma_start(b12_sb[:], bias12[:].rearrange("w b n -> b w n"))
            id_sb = wp.tile([B, B], F32)
            nc.sync.dma_start(id_sb[:], ident[:])
            hist = wp.tile([128, NKC, T * B], BF16)   # gathered h2^T history

            # initial state
            c_st = [sp.tile([B, SL], F32, tag=f"c{l}", name=f"cst{l}") for l in range(NL)]
            for l in range(NL):
                nc.sync.dma_start(c_st[l][:], c0loc[l])
            stage = sp.tile([128, NL, B], BF16)       # AG staging: main rows
            stage_t = sp.tile([16, NL, B], BF16)      # AG staging: 16-row tail
            nc.sync.dma_start(stage[:], h0s[:, 0:128, :].rearrange("l p b -> p l b"))
            nc.sync.dma_start(stage_t[:], h0s[:, 128:144, :].rearrange("l p b -> p l b"))
            hT = [sp.tile([128, 8, B], BF16, tag=f"hT{l}", name=f"hTi{l}") for l in range(NL)]
            c8T = sp.tile([128, NL, B], BF16, tag="c8T")
            for l in range(NL):
                nc.sync.dma_start(
                    hT[l][:], h0full[l, 0:1024, :].rearrange("(c p) b -> p c b", p=128))
            nc.sync.dma_start(c8T[:], h0full[:, 1024:1152, :].rearrange("l p b -> p l b"))

            # ---- phase 1: embedding gather + transpose + P0 ----
            idx_sb = sp.tile([128, 24], I32)
            nc.sync.dma_start(idx_sb[:], x_idx[:].rearrange("(o p) -> p o", p=128))
            embd = dram.tile([T * B, EP], BF16)
            with tc.tile_pool(name="embp", bufs=2) as ep:
                for j in range(24):
                    emb_f = ep.tile([128, EP], F32, tag="embf")
                    nc.gpsimd.memset(emb_f[:, E:EP], 0.0)
                    nc.gpsimd.indirect_dma_start(
                        out=emb_f[:, 0:E], out_offset=None, in_=embw[:],
                        in_offset=bass.IndirectOffsetOnAxis(ap=idx_sb[:, j:j + 1], axis=0))
                    nc.gpsimd.memset(emb_f[:, E:E + 1], 1.0)
                    emb_b = ep.tile([128, EP], BF16, tag="embb")
                    nc.vector.tensor_copy(emb_b[:], emb_f[:])
                    nc.sync.dma_start(
                        embd[:].rearrange("(o p) e -> p o e", p=128)[:, j, :], emb_b[:])
            p0d = dram.tile([T * B, GS], F32)
            with tc.tile_pool(name="embT", bufs=1) as et:
                embT = et.tile([128, 4, T * B], BF16)
                for c in range(4):
                    nc.sync.dma_start_transpose(
                        embT[:, c, :], embd[:, c * 128:(c + 1) * 128])
                for j in range(32):  # 96-token M-tiles
                    psa = pp.tile([96, 512], F32, tag="pA", name="p0a")
                    psb = pp.tile([96, 64], F32, tag="pB", name="p0b")
                    for c in range(4):
                        lhs = embT[:, c, j * 96:(j + 1) * 96]
                        nc.tensor.matmul(psa[:], lhs, wi0_sb[:, c, 0:512],
                                         start=(c == 0), stop=(c == 3))
                        nc.tensor.matmul(psb[:], lhs, wi0_sb[:, c, 512:GS],
                                         start=(c == 0), stop=(c == 3))
                    p0sb = wk.tile([96, GS], F32, tag="p0sb")
                    nc.vector.tensor_copy(p0sb[:, 0:512], psa[:])
                    nc.vector.tensor_copy(p0sb[:, 512:GS], psb[:])
                    nc.sync.dma_start(p0d[j * 96:(j + 1) * 96, :], p0sb[:])

            # ---- decoder pair queue ----
            dec_pairs = []
            for mb in range(4):
                for n in range(DEC_NB):
                    for m in range(mb * DEC_MB, (mb + 1) * DEC_MB):
                        dec_pairs.append((m, n))
            dec_i = 0
            dec_done = [False] * len(dec_pairs)
            wdec_sb = {"n": -1, "tile": None}

            def emit_dec_pair(m, n):
                if wdec_sb["n"] != n or wdec_sb["tile"] is None:
                    wt = wk.tile([128, NKC, 500], BF16, tag="wdec", name="wt")
                    nc.sync.dma_start(
                        wt[:], wdec[:, n * 500:(n + 1) * 500]
                        .rearrange("(c p) v -> p c v", p=128))
                    bt = wk.tile([128, 500], F32, tag="bdect", name="bt")
                    nc.sync.dma_start(bt[:], bdec[:, n * 500:(n + 1) * 500])
                    wdec_sb["n"], wdec_sb["tile"] = n, (wt, bt)
                wt, bt = wdec_sb["tile"]
                psd = ppd.tile([128, 500], F32, tag="pdec")
                for c in range(NKC):
                    nc.tensor.matmul(psd[:], hist[:, c, m * 128:(m + 1) * 128],
                                     wt[:, c, :], start=(c == 0), stop=(c == NKC - 1))
                osb = wk.tile([128, 500], F32, tag="odec")
                nc.vector.tensor_tensor(
                    osb[:], psd[:], bt[:], mybir.AluOpType.add)
                nc.gpsimd.dma_start(
                    dec_out[m * 128:(m + 1) * 128, n * 500:(n + 1) * 500], osb[:])

            # ---- phase 2: wavefront ----
            for s in range(NSS):
                # decoder fill (history complete through t = s-3)
                budget = 6
                scan = dec_i
                while scan < len(dec_pairs) and budget > 0:
                    m, n = dec_pairs[scan]
                    if dec_done[scan] or (m * 128 + 127) // B > s - 3:
                        scan += 1
                        continue
                    emit_dec_pair(m, n)
                    dec_done[scan] = True
                    budget -= 1
                    scan += 1
                while dec_i < len(dec_pairs) and dec_done[dec_i]:
                    dec_i += 1

                for l in range(NL):
                    t = s - l
                    if not (0 <= t < T):
                        continue
                    psa = pp.tile([B, 512], F32, tag="pA")
                    psb = pp.tile([B, 64], F32, tag="pB")
                    if l == 0:
                        srcs = [(0, hT[0], c8T, 0)]
                    else:
                        srcs = [(2 * l - 1, hT[l - 1], c8T, l - 1),
                                (2 * l, hT[l], c8T, l)]
                    for wslot, ht_t, c8_t, c8l in srcs:
                        for c in range(NKC):
                            lhs = ht_t[:, c, :] if c < 8 else c8_t[:, c8l, :]
                            rhs = wrec_sb[:, wslot, c, :]
                            st = (wslot == srcs[0][0]) and (c == 0)
                            last = (wslot == srcs[-1][0]) and (c == NKC - 1)
                            nc.tensor.matmul(
                                psa[:], lhs, rhs[:, 0:512], start=st, stop=last)
                            nc.tensor.matmul(
                                psb[:], lhs, rhs[:, 512:GS], start=st, stop=last)
                    # p_sb = psum + (P0[t] | bias)
                    p_sb = wk.tile([B, GS], F32, tag="psb")
                    if l == 0:
                        p0t = wk.tile([B, GS], F32, tag="p0t")
                        nc.gpsimd.dma_start(p0t[:], p0d[t * B:(t + 1) * B, :])
                        add_src = p0t[:]
                    else:
                        add_src = b12_sb[:, l - 1, :]
                    nc.vector.tensor_tensor(p_sb[:, 0:512], psa[:],
                                            add_src[:, 0:512], mybir.AluOpType.add)
                    nc.vector.tensor_tensor(p_sb[:, 512:GS], psb[:],
                                            add_src[:, 512:GS], mybir.AluOpType.add)
                    # gates: [i | f | o | g] each SL wide
                    ifo = wk.tile([B, 3 * SL], F32, tag="ifo")
                    g_t = wk.tile([B, SL], F32, tag="gg")
                    nc.scalar.activation(ifo[:], p_sb[:, 0:3 * SL], AF.Sigmoid)
                    nc.scalar.activation(g_t[:], p_sb[:, 3 * SL:GS], AF.Tanh)
                    c_new = sp.tile([B, SL], F32, tag=f"c{l}", name=f"cnew{l}")
                    t1 = wk.tile([B, SL], F32, tag="t1")
                    nc.vector.tensor_tensor(t1[:], ifo[:, SL:2 * SL], c_st[l][:],
                                            mybir.AluOpType.mult)
                    t2 = wk.tile([B, SL], F32, tag="t2")
                    nc.vector.tensor_tensor(t2[:], ifo[:, 0:SL], g_t[:],
                                            mybir.AluOpType.mult)
                    nc.vector.tensor_tensor(c_new[:], t1[:], t2[:],
                                            mybir.AluOpType.add)
                    c_st[l] = c_new
                    tc_t = wk.tile([B, SL], F32, tag="tcx")
                    nc.scalar.activation(tc_t[:], c_new[:], AF.Tanh)
                    h_new = wk.tile([B, SL], F32, tag="hn")
                    nc.vector.tensor_tensor(h_new[:], ifo[:, 2 * SL:3 * SL], tc_t[:],
                                            mybir.AluOpType.mult)
                    if t == T - 1:
                        nc.sync.dma_start(h_out[l], h_new[:])
                        nc.sync.dma_start(c_out[l], c_new[:])
                    # transpose own slice -> staging
                    ptr = ppt.tile([128, 2, B], F32, tag="ptr")
                    nc.tensor.transpose(ptr[:, 0, :], h_new[:, 0:128], id_sb[:])
                    nc.tensor.transpose(ptr[0:16, 1, :], h_new[:, 128:SL], id_sb[:])
                    nc.vector.tensor_copy(stage[:, l, :], ptr[:, 0, :])
                    nc.vector.tensor_copy(stage_t[:, l, :], ptr[0:16, 1, :])

                # ---- exchange ----
                agin = dram.tile([NL * SL, B], BF16, tag="agin")
                nc.sync.dma_start(
                    agin[:].rearrange("(l p) b -> l p b", p=SL)[:, 0:128, :]
                    .rearrange("l p b -> p l b"), stage[:])
                nc.sync.dma_start(
                    agin[:].rearrange("(l p) b -> l p b", p=SL)[:, 128:SL, :]
                    .rearrange("l p b -> p l b"), stage_t[:])
                agout = dram.tile([NC * NL * SL, B], BF16, tag="agout")
                nc.gpsimd.collective_compute(
                    "AllGather", mybir.AluOpType.bypass,
                    replica_groups=[list(range(NC))],
                    ins=[agin.opt()], outs=[agout.opt()])
                agv = agout[:].rearrange("(r l p) b -> r l p b", l=NL, p=SL)
                hT = [sp.tile([128, 8, B], BF16, tag=f"hT{l}", name=f"hTn{l}") for l in range(NL)]
                for l in range(NL):
                    nc.sync.dma_start(
                        hT[l][:], agv[:, l, 0:128, :].rearrange("r p b -> p r b"))
                c8s = wk.tile([16, NC, NL, B], BF16, tag="c8s")
                nc.scalar.dma_start(c8s[:], agv[:, :, 128:SL, :].rearrange("r l p b -> p r l b"))
                c8T = sp.tile([128, NL, B], BF16, tag="c8T")
                for r in range(NC):
                    nc.scalar.dma_start(c8T[16 * r:16 * (r + 1), :, :], c8s[:, r, :, :])
                # h2 history append (h2(t2) for t2 = s-2)
                t2 = s - 2
                if 0 <= t2 < T:
                    nc.scalar.dma_start(
                        hist[:, 0:8, t2 * B:(t2 + 1) * B],
                        agv[:, 2, 0:128, :].rearrange("r p b -> p r b"))
                    nc.scalar.dma_start(hist[:, 8, t2 * B:(t2 + 1) * B], c8T[:, 2, :])

            # ---- decoder tail ----
            for i2 in range(len(dec_pairs)):
                if not dec_done[i2]:
                    emit_dec_pair(*dec_pairs[i2])
                    dec_done[i2] = True

    split_excess_waits(nc)
    return di


def _prep_inputs(r, x, h0, c0, emb_W, W_i0, b_i0, W_h0, b_h0,
                 W_i1, b_i1, W_h1, b_h1, W_i2, b_i2, W_h2, b_h2, W_dec, b_dec):
    bf = ml_dtypes.bfloat16
    gr = _gate_rows(r)
    d = _own_dims(r)

    wi0 = np.zeros((EP, GS), np.float32)
    wi0[0:E, :] = W_i0[gr, :].T
    wi0[E, :] = b_i0[gr] + b_h0[gr]

    wrec = np.stack([W_h0[gr, :].T, W_i1[gr, :].T, W_h1[gr, :].T,
                     W_i2[gr, :].T, W_h2[gr, :].T])  # [5, H, GS]

    b1 = (b_i1[gr] + b_h1[gr]).astype(np.float32)
    b2 = (b_i2[gr] + b_h2[gr]).astype(np.float32)
    bias12 = np.stack([np.tile(b1, (B, 1)), np.tile(b2, (B, 1))])

    vs = slice(r * VS, (r + 1) * VS)
    wdec = W_dec[vs, :].T.astype(bf)
    bdec = np.tile(b_dec[vs].astype(np.float32), (128, 1))

    h0T = np.transpose(h0, (0, 2, 1))  # [NL, H, B]
    h0s = h0T[:, d, :].astype(bf)      # [NL, 144, B]
    return {
        "x_idx": x.reshape(-1).astype(np.int32),
        "embw": emb_W.astype(np.float32),
        "wi0": wi0.astype(bf),
        "wrec": wrec.astype(bf),
        "bias12": bias12.astype(np.float32),
        "wdec": wdec,
        "bdec": bdec.astype(np.float32),
        "ident": np.eye(B, dtype=np.float32),
        "h0s": h0s,
        "c0loc": c0[:, :, d].astype(np.float32),
        "h0full": h0T.astype(bf),
    }


def _enable_axon_profiling():
    """Register the NTFF hook (this image's antenv lacks axon_hooks) and stub
    the artifact upload so trace=True works locally."""
    import sys
    import types

    import concourse.bass_utils as bu
    try:
        from trn_agent_boot.trn_boot import _ntff_profile_via_ctypes
    except ImportError:
        return
    hook = _ntff_profile_via_ctypes("/opt/axon/libaxon_pjrt.so")
    mod = types.ModuleType("antenv.axon_hooks")
    mod.get_axon_ntff_profile_hook = lambda: hook
    mod.set_axon_ntff_profile_hook = lambda h: None
    sys.modules["antenv.axon_hooks"] = mod
    bu.upload_artifacts = lambda tmpdir: "file://" + str(tmpdir)


_CACHE = {}


def kernel(**inputs):
    inputs = {k: np.asarray(v) for k, v in inputs.items()}
    x = inputs["x"].astype(np.int32)
    h0, c0 = inputs["h0"], inputs["c0"]

    if "nc" not in _CACHE:
        nc = bass.Bass("TRN2", target_bir_lowering=False, debug=False,
                       num_devices=NC)
        build(nc)
        _CACHE["nc"] = nc
    nc = _CACHE["nc"]

    in_maps = [_prep_inputs(r, x=x, **{k: v for k, v in inputs.items()
                                       if k not in ("x", "h0", "c0")},
                            h0=h0, c0=c0) for r in range(NC)]

    trace = bool(int(os.environ.get("BASS_LSTM_TRACE", "0")))
    if trace:
        _enable_axon_profiling()
    res = run_bass_kernel_spmd(nc, in_maps, core_ids=list(range(NC)),
                               trace=trace)
    if trace and res.exec_time_ns is not None:
        print(f"HW exec time: {res.exec_time_ns} ns")
        _CACHE["exec_time_ns"] = res.exec_time_ns

    decoded = np.empty((T * B, V), np.float32)
    h_fin = np.empty((NL, B, H), np.float32)
    c_fin = np.empty((NL, B, H), np.float32)
    for r in range(NC):
        out = res.results[r]
        decoded[:, r * VS:(r + 1) * VS] = out["dec_out"]
        d = _own_dims(r)
        h_fin[:, :, d] = out["h_out"]
        c_fin[:, :, d] = out["c_out"]
    return decoded.reshape(T, B, V), (h_fin, c_fin)


# revision 9
# speedup vs baseline: 1.0433x; 1.0356x over previous
"""AWD-LSTM (3-layer, T=64, B=48, H=1152, V=32000) on 8 TRN2 NeuronCores.

Sharding: tensor-parallel over the 4H gate dim for the recurrence (each core
owns 144 h-dims = 128-aligned chunk r plus a 16-row piece of chunk 8, so the
gathered transposed hidden state lands on 128-partition tiles with no
repacking), vocab-parallel (4000/core) for the decoder.  One ncfw AllGather
of the three layers' transposed h-slices per wavefront super-step; the
decoder is interleaved into the AllGather latency gaps.  LSTM weights are
SBUF-resident in bf16; cell state and gate math in fp32.
"""

import os

import ml_dtypes
import numpy as np

import concourse.bass as bass
import concourse.mybir as mybir
import concourse.tile as tile
from concourse.bass_utils import run_bass_kernel_spmd
from concourse.vector_clock import ScopedClock

V, E, H, NL = 32000, 400, 1152, 3
T, B = 64, 48
NC = 8
SL = H // NC            # 144 h-dims owned per core
GS = 4 * SL             # 576 gate dims per core
EP = 512                # padded embedding width (bias lane at col 400)
VS = V // NC            # 4000 vocab rows per core
NSS = T + NL - 1        # 66 wavefront super-steps
NKC = H // 128          # 9 K-chunks
DEC_NB, DEC_MB = 8, 6   # decoder: 8 vocab chunks of 500; m-tiles in blocks of 6

F32, BF16, I32 = mybir.dt.float32, mybir.dt.bfloat16, mybir.dt.int32
AF = mybir.ActivationFunctionType


class PatchedTC(tile.TileContext):
    """This walrus build folds at most one sync wait per instruction; split
    the tile-exit drain's waits onto one SP nop each."""

    def _drain_and_barrier(self, tick_clock, wait_clock):
        gc = tick_clock.global_clock
        for i in range(len(gc)):
            if gc[i] > 0:
                nop = self.nc.sync.nop(nofuse=True, hint=f"drain_split_{i}")
                pc = ScopedClock()
                pc.require_at_least(None, i, gc[i])
                wait_clock.add_sem_waits(nop.ins, pc)
        self.nc.sync.drain()
        self.nc.all_engine_barrier()
        popped = self.nc._tile_sem_poison_stack.pop()
        assert popped is self._sem_poison
        self.nc.clear_and_free_semaphores(list(self.sems.allocated().values()))
        self.nc.all_engine_barrier()


def split_excess_waits(nc, max_waits=1):
    """Hoist sync waits beyond max_waits onto preceding same-engine nops."""
    for f in nc.m.functions:
        for bb in f.blocks:
            new_insts = []
            for inst in bb.instructions:
                si = getattr(inst, "sync_info", None)
                if si is not None and si.on_wait and len(si.on_wait) > max_waits:
                    extra = si.on_wait[max_waits:]
                    si.on_wait = si.on_wait[:max_waits]
                    for j in range(0, len(extra), max_waits):
                        new_insts.append(mybir.InstNoOp(
                            name=f"{inst.name}_ws{j}", engine=inst.engine,
                            ins=[], outs=[],
                            sync_info=mybir.SyncInfo(
                                on_wait=extra[j:j + max_waits], on_update=[]),
                        ))
                new_insts.append(inst)
            bb.instructions[:] = new_insts


def _own_dims(r):
    return list(range(128 * r, 128 * r + 128)) + \
        list(range(1024 + 16 * r, 1024 + 16 * r + 16))


def _gate_rows(r):
    d = _own_dims(r)
    return [g * H + i for g in range(4) for i in d]


def build(nc):
    di = {}  # dram tensors (inputs)
    def inp(name, shape, dt):
        di[name] = nc.dram_tensor(name, shape, dt, kind="ExternalInput")
        return di[name]

    x_idx = inp("x_idx", [T * B], I32)
    embw = inp("embw", [V, E], F32)
    wi0 = inp("wi0", [EP, GS], BF16)                  # padded, bias at row 400
    wrec = inp("wrec", [5, H, GS], BF16)              # Wh0,Wi1,Wh1,Wi2,Wh2 (transposed slices)
    bias12 = inp("bias12", [2, B, GS], F32)           # replicated bias tiles l1,l2
    wdec = inp("wdec", [H, VS], BF16)
    bdec = inp("bdec", [128, VS], F32)
    ident = inp("ident", [B, B], F32)
    h0s = inp("h0s", [NL, 128 + 16, B], BF16)         # own transposed h0 slices (main+tail rows)
    c0loc = inp("c0loc", [NL, B, SL], F32)
    h0full = inp("h0full", [NL, H, B], BF16)          # full transposed h0

    dec_out = nc.dram_tensor("dec_out", [T * B, VS], F32, kind="ExternalOutput")
    h_out = nc.dram_tensor("h_out", [NL, B, SL], F32, kind="ExternalOutput")
    c_out = nc.dram_tensor("c_out", [NL, B, SL], F32, kind="ExternalOutput")

    with PatchedTC(nc) as tc:
        with tc.tile_pool(name="wpool", bufs=1) as wp, \
             tc.tile_pool(name="spool", bufs=2) as sp, \
             tc.tile_pool(name="work", bufs=2) as wk, \
             tc.tile_pool(name="psum", bufs=2, space="PSUM") as pp, \
             tc.tile_pool(name="psumt", bufs=2, space="PSUM") as ppt, \
             tc.tile_pool(name="psumd", bufs=1, space="PSUM") as ppd, \
             tc.tile_pool(name="dram", bufs=2, space="DRAM") as dram:

            # ---- persistent SBUF ----
            wi0_sb = wp.tile([128, 4, GS], BF16)
            nc.sync.dma_start(wi0_sb[:], wi0[:].rearrange("(c p) n -> p c n", p=128))
            wrec_sb = wp.tile([128, 5, NKC, GS], BF16)
            nc.sync.dma_start(
                wrec_sb[:], wrec[:].rearrange("w (c p) n -> p w c n", p=128))
            b12_sb = wp.tile([B, 2, GS], F32)
            nc.sync.dma_start(b12_sb[:], bias12[:].rearrange("w b n -> b w n"))
            id_sb = wp.tile([B, B], F32)
            nc.sync.dma_start(id_sb[:], ident[:])
            hist = wp.tile([128, NKC, T * B], BF16)   # gathered h2^T history

            # initial state
            c_st = [sp.tile([B, SL], F32, tag=f"c{l}", name=f"cst{l}") for l in range(NL)]
            for l in range(NL):
                nc.sync.dma_start(c_st[l][:], c0loc[l])
            stage = sp.tile([128, NL, B], BF16)       # AG staging: main rows
            stage_t = sp.tile([16, NL, B], BF16)      # AG staging: 16-row tail
            nc.sync.dma_start(stage[:], h0s[:, 0:128, :].rearrange("l p b -> p l b"))
            nc.sync.dma_start(stage_t[:], h0s[:, 128:144, :].rearrange("l p b -> p l b"))
            hT = [sp.tile([128, 8, B], BF16, tag=f"hT{l}", name=f"hTi{l}") for l in range(NL)]
            c8T = sp.tile([128, NL, B], BF16, tag="c8T")
            for l in range(NL):
                nc.sync.dma_start(
                    hT[l][:], h0full[l, 0:1024, :].rearrange("(c p) b -> p c b", p=128))
            nc.sync.dma_start(c8T[:], h0full[:, 1024:1152, :].rearrange("l p b -> p l b"))

            # ---- phase 1: embedding gather + transpose + P0 ----
            idx_sb = sp.tile([128, 24], I32)
            nc.sync.dma_start(idx_sb[:], x_idx[:].rearrange("(o p) -> p o", p=128))
            embd = dram.tile([T * B, EP], BF16)
            with tc.tile_pool(name="embp", bufs=2) as ep:
                for j in range(24):
                    emb_f = ep.tile([128, EP], F32, tag="embf")
                    nc.gpsimd.memset(emb_f[:, E:EP], 0.0)
                    nc.gpsimd.indirect_dma_start(
                        out=emb_f[:, 0:E], out_offset=None, in_=embw[:],
                        in_offset=bass.IndirectOffsetOnAxis(ap=idx_sb[:, j:j + 1], axis=0))
                    nc.gpsimd.memset(emb_f[:, E:E + 1], 1.0)
                    emb_b = ep.tile([128, EP], BF16, tag="embb")
                    nc.vector.tensor_copy(emb_b[:], emb_f[:])
                    nc.sync.dma_start(
                        embd[:].rearrange("(o p) e -> p o e", p=128)[:, j, :], emb_b[:])
            p0d = dram.tile([T * B, GS], F32)
            with tc.tile_pool(name="embT", bufs=1) as et:
                embT = et.tile([128, 4, T * B], BF16)
                for c in range(4):
                    nc.sync.dma_start_transpose(
                        embT[:, c, :], embd[:, c * 128:(c + 1) * 128])
                for j in range(32):  # 96-token M-tiles
                    psa = pp.tile([96, 512], F32, tag="pA", name="p0a")
                    psb = pp.tile([96, 64], F32, tag="pB", name="p0b")
                    for c in range(4):
                        lhs = embT[:, c, j * 96:(j + 1) * 96]
                        nc.tensor.matmul(psa[:], lhs, wi0_sb[:, c, 0:512],
                                         start=(c == 0), stop=(c == 3))
                        nc.tensor.matmul(psb[:], lhs, wi0_sb[:, c, 512:GS],
                                         start=(c == 0), stop=(c == 3))
                    p0sb = wk.tile([96, GS], F32, tag="p0sb")
                    nc.vector.tensor_copy(p0sb[:, 0:512], psa[:])
                    nc.vector.tensor_copy(p0sb[:, 512:GS], psb[:])
                    nc.sync.dma_start(p0d[j * 96:(j + 1) * 96, :], p0sb[:])

            # ---- decoder pair queue ----
            dec_pairs = []
            for mb in range(4):
                for n in range(DEC_NB):
                    for m in range(mb * DEC_MB, (mb + 1) * DEC_MB):
                        dec_pairs.append((m, n))
            dec_i = 0
            dec_done = [False] * len(dec_pairs)
            wdec_sb = {"n": -1, "tile": None}

            def emit_dec_pair(m, n):
                if wdec_sb["n"] != n or wdec_sb["tile"] is None:
                    wt = wk.tile([128, NKC, 500], BF16, tag="wdec", name="wt")
                    nc.sync.dma_start(
                        wt[:], wdec[:, n * 500:(n + 1) * 500]
                        .rearrange("(c p) v -> p c v", p=128))
                    bt = wk.tile([128, 500], F32, tag="bdect", name="bt")
                    nc.sync.dma_start(bt[:], bdec[:, n * 500:(n + 1) * 500])
                    wdec_sb["n"], wdec_sb["tile"] = n, (wt, bt)
                wt, bt = wdec_sb["tile"]
                psd = ppd.tile([128, 500], F32, tag="pdec")
                for c in range(NKC):
                    nc.tensor.matmul(psd[:], hist[:, c, m * 128:(m + 1) * 128],
                                     wt[:, c, :], start=(c == 0), stop=(c == NKC - 1))
                osb = wk.tile([128, 500], F32, tag="odec")
                nc.vector.tensor_tensor(
                    osb[:], psd[:], bt[:], mybir.AluOpType.add)
                nc.gpsimd.dma_start(
                    dec_out[m * 128:(m + 1) * 128, n * 500:(n + 1) * 500], osb[:])

            # ---- phase 2: wavefront ----
            for s in range(NSS):
                for l in range(NL):
                    t = s - l
                    if not (0 <= t < T):
                        continue
                    psa = pp.tile([B, 512], F32, tag="pA")
                    psb = pp.tile([B, 64], F32, tag="pB")
                    if l == 0:
                        srcs = [(0, hT[0], c8T, 0)]
                    else:
                        srcs = [(2 * l - 1, hT[l - 1], c8T, l - 1),
                                (2 * l, hT[l], c8T, l)]
                    for wslot, ht_t, c8_t, c8l in srcs:
                        for c in range(NKC):
                            lhs = ht_t[:, c, :] if c < 8 else c8_t[:, c8l, :]
                            rhs = wrec_sb[:, wslot, c, :]
                            st = (wslot == srcs[0][0]) and (c == 0)
                            last = (wslot == srcs[-1][0]) and (c == NKC - 1)
                            nc.tensor.matmul(
                                psa[:], lhs, rhs[:, 0:512], start=st, stop=last)
                            nc.tensor.matmul(
                                psb[:], lhs, rhs[:, 512:GS], start=st, stop=last)
                    # p_sb = psum + (P0[t] | bias)
                    p_sb = wk.tile([B, GS], F32, tag="psb")
                    if l == 0:
                        p0t = wk.tile([B, GS], F32, tag="p0t")
                        nc.gpsimd.dma_start(p0t[:], p0d[t * B:(t + 1) * B, :])
                        add_src = p0t[:]
                    else:
                        add_src = b12_sb[:, l - 1, :]
                    nc.vector.tensor_tensor(p_sb[:, 0:512], psa[:],
                                            add_src[:, 0:512], mybir.AluOpType.add)
                    nc.vector.tensor_tensor(p_sb[:, 512:GS], psb[:],
                                            add_src[:, 512:GS], mybir.AluOpType.add)
                    # gates: [i | f | o | g] each SL wide
                    ifo = wk.tile([B, 3 * SL], F32, tag="ifo")
                    g_t = wk.tile([B, SL], F32, tag="gg")
                    nc.scalar.activation(ifo[:], p_sb[:, 0:3 * SL], AF.Sigmoid)
                    nc.scalar.activation(g_t[:], p_sb[:, 3 * SL:GS], AF.Tanh)
                    c_new = sp.tile([B, SL], F32, tag=f"c{l}", name=f"cnew{l}")
                    t1 = wk.tile([B, SL], F32, tag="t1")
                    nc.vector.tensor_tensor(t1[:], ifo[:, SL:2 * SL], c_st[l][:],
                                            mybir.AluOpType.mult)
                    t2 = wk.tile([B, SL], F32, tag="t2")
                    nc.vector.tensor_tensor(t2[:], ifo[:, 0:SL], g_t[:],
                                            mybir.AluOpType.mult)
                    nc.vector.tensor_tensor(c_new[:], t1[:], t2[:],
                                            mybir.AluOpType.add)
                    c_st[l] = c_new
                    tc_t = wk.tile([B, SL], F32, tag="tcx")
                    nc.scalar.activation(tc_t[:], c_new[:], AF.Tanh)
                    h_new = wk.tile([B, SL], F32, tag="hn")
                    nc.vector.tensor_tensor(h_new[:], ifo[:, 2 * SL:3 * SL], tc_t[:],
                                            mybir.AluOpType.mult)
                    if t == T - 1:
                        nc.sync.dma_start(h_out[l], h_new[:])
                        nc.sync.dma_start(c_out[l], c_new[:])
                    # transpose own slice -> staging
                    ptr = ppt.tile([128, 2, B], F32, tag="ptr")
                    nc.tensor.transpose(ptr[:, 0, :], h_new[:, 0:128], id_sb[:])
                    nc.tensor.transpose(ptr[0:16, 1, :], h_new[:, 128:SL], id_sb[:])
                    nc.vector.tensor_copy(stage[:, l, :], ptr[:, 0, :])
                    nc.vector.tensor_copy(stage_t[:, l, :], ptr[0:16, 1, :])

                # ---- exchange ----
                agin = dram.tile([NL * SL, B], BF16, tag="agin")
                nc.sync.dma_start(
                    agin[:].rearrange("(l p) b -> l p b", p=SL)[:, 0:128, :]
                    .rearrange("l p b -> p l b"), stage[:])
                nc.sync.dma_start(
                    agin[:].rearrange("(l p) b -> l p b", p=SL)[:, 128:SL, :]
                    .rearrange("l p b -> p l b"), stage_t[:])
                agout = dram.tile([NC * NL * SL, B], BF16, tag="agout")
                nc.gpsimd.collective_compute(
                    "AllGather", mybir.AluOpType.bypass,
                    replica_groups=[list(range(NC))],
                    ins=[agin.opt()], outs=[agout.opt()])
                agv = agout[:].rearrange("(r l p) b -> r l p b", l=NL, p=SL)
                hT = [sp.tile([128, 8, B], BF16, tag=f"hT{l}", name=f"hTn{l}") for l in range(NL)]
                for l in range(NL):
                    nc.sync.dma_start(
                        hT[l][:], agv[:, l, 0:128, :].rearrange("r p b -> p r b"))
                c8s = wk.tile([16, NC, NL, B], BF16, tag="c8s")
                nc.scalar.dma_start(c8s[:], agv[:, :, 128:SL, :].rearrange("r l p b -> p r l b"))
                c8T = sp.tile([128, NL, B], BF16, tag="c8T")
                for r in range(NC):
                    nc.scalar.dma_start(c8T[16 * r:16 * (r + 1), :, :], c8s[:, r, :, :])
                # h2 history append (h2(t2) for t2 = s-2)
                t2 = s - 2
                if 0 <= t2 < T:
                    nc.scalar.dma_start(
                        hist[:, 0:8, t2 * B:(t2 + 1) * B],
                        agv[:, 2, 0:128, :].rearrange("r p b -> p r b"))
                    nc.scalar.dma_start(hist[:, 8, t2 * B:(t2 + 1) * B], c8T[:, 2, :])

                # decoder fill (history complete through t = s-3)
                budget = 6
                scan = dec_i
                while scan < len(dec_pairs) and budget > 0:
                    m, n = dec_pairs[scan]
                    if dec_done[scan] or (m * 128 + 127) // B > s - 3:
                        scan += 1
                        continue
                    emit_dec_pair(m, n)
                    dec_done[scan] = True
                    budget -= 1
                    scan += 1
                while dec_i < len(dec_pairs) and dec_done[dec_i]:
                    dec_i += 1


            # ---- decoder tail ----
            for i2 in range(len(dec_pairs)):
                if not dec_done[i2]:
                    emit_dec_pair(*dec_pairs[i2])
                    dec_done[i2] = True

    split_excess_waits(nc)
    return di


def _prep_inputs(r, x, h0, c0, emb_W, W_i0, b_i0, W_h0, b_h0,
                 W_i1, b_i1, W_h1, b_h1, W_i2, b_i2, W_h2, b_h2, W_dec, b_dec):
    bf = ml_dtypes.bfloat16
    gr = _gate_rows(r)
    d = _own_dims(r)

    wi0 = np.zeros((EP, GS), np.float32)
    wi0[0:E, :] = W_i0[gr, :].T
    wi0[E, :] = b_i0[gr] + b_h0[gr]

    wrec = np.stack([W_h0[gr, :].T, W_i1[gr, :].T, W_h1[gr, :].T,
                     W_i2[gr, :].T, W_h2[gr, :].T])  # [5, H, GS]

    b1 = (b_i1[gr] + b_h1[gr]).astype(np.float32)
    b2 = (b_i2[gr] + b_h2[gr]).astype(np.float32)
    bias12 = np.stack([np.tile(b1, (B, 1)), np.tile(b2, (B, 1))])

    vs = slice(r * VS, (r + 1) * VS)
    wdec = W_dec[vs, :].T.astype(bf)
    bdec = np.tile(b_dec[vs].astype(np.float32), (128, 1))

    h0T = np.transpose(h0, (0, 2, 1))  # [NL, H, B]
    h0s = h0T[:, d, :].astype(bf)      # [NL, 144, B]
    return {
        "x_idx": x.reshape(-1).astype(np.int32),
        "embw": emb_W.astype(np.float32),
        "wi0": wi0.astype(bf),
        "wrec": wrec.astype(bf),
        "bias12": bias12.astype(np.float32),
        "wdec": wdec,
        "bdec": bdec.astype(np.float32),
        "ident": np.eye(B, dtype=np.float32),
        "h0s": h0s,
        "c0loc": c0[:, :, d].astype(np.float32),
        "h0full": h0T.astype(bf),
    }


def _enable_axon_profiling():
    """Register the NTFF hook (this image's antenv lacks axon_hooks) and stub
    the artifact upload so trace=True works locally."""
    import sys
    import types

    import concourse.bass_utils as bu
    try:
        from trn_agent_boot.trn_boot import _ntff_profile_via_ctypes
    except ImportError:
        return
    hook = _ntff_profile_via_ctypes("/opt/axon/libaxon_pjrt.so")
    mod = types.ModuleType("antenv.axon_hooks")
    mod.get_axon_ntff_profile_hook = lambda: hook
    mod.set_axon_ntff_profile_hook = lambda h: None
    sys.modules["antenv.axon_hooks"] = mod
    bu.upload_artifacts = lambda tmpdir: "file://" + str(tmpdir)


_CACHE = {}


def kernel(**inputs):
    inputs = {k: np.asarray(v) for k, v in inputs.items()}
    x = inputs["x"].astype(np.int32)
    h0, c0 = inputs["h0"], inputs["c0"]

    if "nc" not in _CACHE:
        nc = bass.Bass("TRN2", target_bir_lowering=False, debug=False,
                       num_devices=NC)
        build(nc)
        _CACHE["nc"] = nc
    nc = _CACHE["nc"]

    in_maps = [_prep_inputs(r, x=x, **{k: v for k, v in inputs.items()
                                       if k not in ("x", "h0", "c0")},
                            h0=h0, c0=c0) for r in range(NC)]

    trace = bool(int(os.environ.get("BASS_LSTM_TRACE", "0")))
    if trace:
        _enable_axon_profiling()
    res = run_bass_kernel_spmd(nc, in_maps, core_ids=list(range(NC)),
                               trace=trace)
    if trace and res.exec_time_ns is not None:
        print(f"HW exec time: {res.exec_time_ns} ns")
        _CACHE["exec_time_ns"] = res.exec_time_ns

    decoded = np.empty((T * B, V), np.float32)
    h_fin = np.empty((NL, B, H), np.float32)
    c_fin = np.empty((NL, B, H), np.float32)
    for r in range(NC):
        out = res.results[r]
        decoded[:, r * VS:(r + 1) * VS] = out["dec_out"]
        d = _own_dims(r)
        h_fin[:, :, d] = out["h_out"]
        c_fin[:, :, d] = out["c_out"]
    return decoded.reshape(T, B, V), (h_fin, c_fin)
